# revision 13
# baseline (speedup 1.0000x reference)
"""Trainium2 Bass kernel for DynamicSparseAttention.

Reference computation (per batch b, head h):
    scores  = Q @ K^T                      [L, S]
    dense   = softmax(scores, axis=-1)
    routing = dense ** 5
    combined = (routing + dense) * 0.5
    sparse  = combined / sum(combined, -1, keepdims=True)
    out     = sparse @ V                   [L, D]

Math: with p = exp(s - m), Z = sum_s p (any per-row shift m),
    out = (P5 @ V + Z^4 * (P @ V)) / (W + Z^5),  W = sum_s p^5.
A ones-column appended to V makes the matmul accumulators carry the row sums:
    B = P @ [V|1]  -> B[:, D] = Z;   A = P5 @ [V|1] -> A[:, D] = W.

Numerical strategy (two rounds, unconditionally stable):
  Round A uses a loose per-row shift m1 = sqrt(2 ln S)|q_l| + 25 (only needs
  to be within ~+-80 of the true row max).  Its ones-column yields Z_A, i.e.
  the true per-row logsumexp m2 = m1 + ln Z_A.  Round B recomputes scores
  shifted by m2 (so sum exp(s - m2) == 1) and accumulates A = P5 @ [V|1]
  with p5 = exp(5(s - m2)) in [e^-38, 1] — no overflow/underflow possible.
  With Z_2 == 1 the result collapses to
    out = (A + B/Z_A)[:, :D] / (A + B/Z_A)[:, D].

Both shifts are folded into the QK^T matmul as a 65th contraction channel:
  Q_aug = [q | -m],  K_aug = [k | 1]
Round A runs with m = m1; between rounds the shift row is updated in place
to -m2 = -m1 - ln Z_A (ACT Ln on the PSUM accumulator row, then a DVE
subtract, all on partition 64 — the next l-half's columns are untouched),
and round B reruns the same matmul shape with the corrected shift.

Layout: scores are computed transposed, [s partitions, l free], so the
exp() outputs feed the P@V' matmuls directly (contraction over s on
partitions, V' stationary).  Matmuls run as float32r (full-rate fp32).

Sharding: B*H = 32 (b,h) pairs, 4 per core across 8 cores, no cross-core
communication.  kernel() takes full inputs and returns the full output.
"""

import os
import sys
import numpy as np

for _p in ("/opt/trn_rl_repo",):
    if os.path.isdir(_p) and _p not in sys.path:
        sys.path.insert(0, _p)

from contextlib import ExitStack

import json as _json

import concourse.bass as bass
import concourse.mybir as mybir
import concourse.tile as tile
import concourse.bass2jax as _bass2jax
import concourse.bass_utils as _bass_utils
from concourse.bass_utils import run_bass_kernel_spmd
from concourse.masks import make_identity

# ---------------------------------------------------------------------------
# Workaround: this container's walrus build rejects instructions carrying
# more than one sync wait ("Too many sync wait commands", CoreV3GenImpl
# setupSyncWait<...>).  Tile's scheduler freely attaches 2-3 waits per
# instruction (and ~27 on the tail drain).  Rewrite the BIR JSON before
# compilation: excess waits are hoisted onto freshly inserted same-engine
# NoOp instructions placed immediately before the instruction, one wait
# each.  Semantics are unchanged (waits are conjunctive >= conditions and
# engine program order is preserved).
# ---------------------------------------------------------------------------

_MAX_WAITS = 1


def _split_waits_in_bir(bir_json: bytes) -> bytes:
    bir = _json.loads(bir_json)
    n_new = [0]

    def fix_block(bb):
        out = []
        for inst in bb["instructions"]:
            si = inst.get("sync_info") or {}
            waits = si.get("on_wait") or []
            if len(waits) > _MAX_WAITS:
                excess, keep = waits[:-_MAX_WAITS], waits[-_MAX_WAITS:]
                for w in excess:
                    n_new[0] += 1
                    out.append({
                        "debug": inst.get("debug", 0),
                        "engine": inst["engine"],
                        "ins": [],
                        "name": "I-wsplit-%d" % n_new[0],
                        "opcode": "NoOp",
                        "outs": [],
                        "sync_info": {"on_update": [], "on_wait": [w]},
                    })
                si["on_wait"] = keep
            out.append(inst)
        bb["instructions"] = out

    for fn in bir["functions"]:
        for bb in fn["blocks"]:
            fix_block(bb)
    return _json.dumps(bir).encode()


_orig_compile_bir_kernel = _bass_utils.compile_bir_kernel


def _patched_compile_bir_kernel(bir_json, tmpdir, neff_name="file.neff"):
    return _orig_compile_bir_kernel(
        _split_waits_in_bir(bir_json), tmpdir, neff_name=neff_name
    )


_bass_utils.compile_bir_kernel = _patched_compile_bir_kernel
_bass2jax.compile_bir_kernel = _patched_compile_bir_kernel

B, L, S, H, E, D = 2, 2048, 2048, 16, 64, 64
NCORES = 8
NP = (B * H) // NCORES  # pairs per core = 4
EA = E + 1  # q/k channels: 64 data + shift row
DA = D + 1  # v columns: 64 data + ones column
LT = L // 128
ST = S // 128
LHALF = 1024  # l columns per accumulation pass (PSUM capacity)
NCH = 2  # 512-wide matmul chunks per l-half
FACTOR = 5.0

F32 = mybir.dt.float32
F32R = mybir.dt.float32r
EXP = mybir.ActivationFunctionType.Exp
LN = mybir.ActivationFunctionType.Ln

M_COEF = float(np.sqrt(2.0 * np.log(S)))
M_MARGIN = 25.0


def _r(ap):
    return ap.bitcast(F32R)


def _emit(ctx: ExitStack, tc: tile.TileContext, qa, ka, va, outp):
    nc = tc.nc

    const = ctx.enter_context(tc.tile_pool(name="const", bufs=1))
    nat = ctx.enter_context(tc.tile_pool(name="nat", bufs=4))
    big = ctx.enter_context(tc.tile_pool(name="big", bufs=2))
    vpool = ctx.enter_context(tc.tile_pool(name="vp", bufs=2))
    ppool = ctx.enter_context(tc.tile_pool(name="pp", bufs=3))
    eppool = ctx.enter_context(tc.tile_pool(name="ep", bufs=2))
    opool = ctx.enter_context(tc.tile_pool(name="op", bufs=4))
    zpool = ctx.enter_context(tc.tile_pool(name="zp", bufs=2))

    ps_sc = ctx.enter_context(tc.tile_pool(name="ps_sc", bufs=2, space="PSUM"))
    ps_acc = ctx.enter_context(tc.tile_pool(name="ps_acc", bufs=1, space="PSUM"))

    ident = const.tile([128, 128], F32)
    make_identity(nc, ident)
    ident65 = const.tile([DA, DA], F32)
    make_identity(nc, ident65)

    for bh in range(NP):
        # ---- setup: load Q/K tiles, transpose to [EA, L] via PE ----
        qt = big.tile([EA, L], F32R, tag="qt")
        kt = big.tile([EA, S], F32R, tag="kt")
        for t in range(LT):
            qn = nat.tile([128, EA], F32, tag="nat")
            nc.sync.dma_start(out=qn, in_=qa[bh, t * 128:(t + 1) * 128, :])
            qt_ps = ps_sc.tile([EA, 128], F32, tag="sc", name="qt_ps")
            nc.tensor.transpose(qt_ps, qn, ident)
            nc.vector.tensor_copy(qt[:, t * 128:(t + 1) * 128], qt_ps)
        for t in range(ST):
            kn = nat.tile([128, EA], F32, tag="nat")
            nc.sync.dma_start(out=kn, in_=ka[bh, t * 128:(t + 1) * 128, :])
            kt_ps = ps_sc.tile([EA, 128], F32, tag="sc", name="kt_ps")
            nc.tensor.transpose(kt_ps, kn, ident)
            nc.vector.tensor_copy(kt[:, t * 128:(t + 1) * 128], kt_ps)

        vts = []
        for t in range(ST):
            vt = vpool.tile([128, DA], F32R, tag=f"v{t}", name=f"vt{t}")
            nc.sync.dma_start(out=vt, in_=va[bh, t * 128:(t + 1) * 128, :])
            vts.append(vt)

        # ---- main: per l-half, two rounds over all s-tiles ----
        for lh in range(L // LHALF):
            l0 = lh * LHALF
            accb = [ps_acc.tile([DA, 512], F32, tag="b%d" % c, name="accb")
                    for c in range(NCH)]
            acca = [ps_acc.tile([DA, 512], F32, tag="a%d" % c, name="acca")
                    for c in range(NCH)]

            # round A: p = exp(s - m1); accumulate B = P @ [V|1]
            for st in range(ST):
                sc = ps_sc.tile([128, LHALF], F32, tag="sc", name="scA")
                for c in range(NCH):
                    nc.tensor.matmul(
                        sc[:, c * 512:(c + 1) * 512],
                        lhsT=kt[:, st * 128:(st + 1) * 128],
                        rhs=qt[:, l0 + c * 512: l0 + (c + 1) * 512],
                        start=True, stop=True,
                    )
                p = ppool.tile([128, LHALF], F32R, tag="p", name="p")
                nc.scalar.activation(p, sc, EXP, bias=0.0, scale=1.0)
                for c in range(NCH):
                    nc.tensor.matmul(
                        accb[c], lhsT=vts[st],
                        rhs=p[:, c * 512:(c + 1) * 512],
                        start=(st == 0), stop=(st == ST - 1),
                    )

            # mid: shift row of qt (partition 64, this l-half's columns only)
            # updated in place: -m2 = -m1 - ln Z_A
            zrow = zpool.tile([E + 1, LHALF], F32, tag="zrow", name="zrow")
            for c in range(NCH):
                cs = slice(c * 512, (c + 1) * 512)
                nc.scalar.activation(zrow[E:E + 1, cs], accb[c][D:DA, :], LN,
                                     bias=0.0, scale=1.0)
            nc.vector.tensor_sub(qt[E:E + 1, l0:l0 + LHALF],
                                 qt[E:E + 1, l0:l0 + LHALF],
                                 zrow[E:E + 1, :])

            # round B: p5 = exp(5(s - m2)); accumulate A = P5 @ [V|1]
            for st in range(ST):
                sc = ps_sc.tile([128, LHALF], F32, tag="sc", name="scB")
                for c in range(NCH):
                    nc.tensor.matmul(
                        sc[:, c * 512:(c + 1) * 512],
                        lhsT=kt[:, st * 128:(st + 1) * 128],
                        rhs=qt[:, l0 + c * 512: l0 + (c + 1) * 512],
                        start=True, stop=True,
                    )
                p5 = ppool.tile([128, LHALF], F32R, tag="p5", name="p5")
                nc.scalar.activation(p5, sc, EXP, bias=0.0, scale=FACTOR)
                for c in range(NCH):
                    nc.tensor.matmul(
                        acca[c], lhsT=vts[st],
                        rhs=p5[:, c * 512:(c + 1) * 512],
                        start=(st == 0), stop=(st == ST - 1),
                    )

            # ---- epilogue: out = (A + B/Z_A)[:, :D] / (A + B/Z_A)[:, D] ----
            a_sb = eppool.tile([DA, LHALF], F32, tag="a_sb", name="a_sb")
            b_sb = eppool.tile([DA, LHALF], F32, tag="b_sb", name="b_sb")
            for c in range(NCH):
                cs = slice(c * 512, (c + 1) * 512)
                nc.vector.tensor_copy(a_sb[:, cs], acca[c])
                nc.vector.tensor_copy(b_sb[:, cs], accb[c])
            for ch in range(LHALF // 128):
                at_ps = ps_sc.tile([128, DA], F32, tag="sc", name="at_ps")
                bt_ps = ps_sc.tile([128, DA], F32, tag="sc", name="bt_ps")
                nc.tensor.transpose(at_ps, a_sb[:, ch * 128:(ch + 1) * 128],
                                    ident65)
                nc.tensor.transpose(bt_ps, b_sb[:, ch * 128:(ch + 1) * 128],
                                    ident65)
                z = zpool.tile([128, 4], F32, tag="z", name="z")
                nc.vector.reciprocal(z[:, 0:1], bt_ps[:, D:DA])  # 1/Z_A
                n65 = opool.tile([128, DA], F32, tag="n65", name="n65")
                nc.vector.tensor_scalar_mul(n65, bt_ps, z[:, 0:1])
                nc.vector.tensor_add(n65, n65, at_ps)
                nc.vector.reciprocal(z[:, 1:2], n65[:, D:DA])    # 1/den
                ot = opool.tile([128, D], F32, tag="ot", name="ot")
                nc.vector.tensor_scalar_mul(ot, n65[:, 0:D], z[:, 1:2])
                lrow = l0 + ch * 128
                nc.sync.dma_start(out=outp[bh, lrow:lrow + 128, :], in_=ot)


_CACHE = {}


def _build():
    if "nc" in _CACHE:
        return _CACHE["nc"]
    nc = bass.Bass()
    qa = nc.declare_dram_parameter("qa", [NP, L, EA], F32, isOutput=False)
    ka = nc.declare_dram_parameter("ka", [NP, S, EA], F32, isOutput=False)
    va = nc.declare_dram_parameter("va", [NP, S, DA], F32R, isOutput=False)
    outp = nc.declare_dram_parameter("out", [NP, L, D], F32, isOutput=True)
    with tile.TileContext(nc) as tc:
        with ExitStack() as ctx:
            _emit(ctx, tc, qa[:], ka[:], va[:], outp[:])
    _CACHE["nc"] = nc
    return nc


def _prep_inputs(queries, keys, values):
    q = np.ascontiguousarray(np.asarray(queries, np.float32).transpose(0, 2, 1, 3)
                             ).reshape(B * H, L, E)
    k = np.ascontiguousarray(np.asarray(keys, np.float32).transpose(0, 2, 1, 3)
                             ).reshape(B * H, S, E)
    v = np.ascontiguousarray(np.asarray(values, np.float32).transpose(0, 2, 1, 3)
                             ).reshape(B * H, S, D)
    m1 = (M_COEF * np.sqrt((q.astype(np.float64) ** 2).sum(-1)) + M_MARGIN
          ).astype(np.float32)  # [BH, L]
    one = np.ones((B * H, S, 1), np.float32)
    qa = np.concatenate([q, -m1[..., None]], axis=-1)
    ka = np.concatenate([k, one], axis=-1)
    va = np.concatenate([v, one], axis=-1)
    in_maps = []
    for c in range(NCORES):
        sl = slice(c * NP, (c + 1) * NP)
        in_maps.append({
            "qa": np.ascontiguousarray(qa[sl]),
            "ka": np.ascontiguousarray(ka[sl]),
            "va": np.ascontiguousarray(va[sl]),
        })
    return in_maps


def _gather(results):
    outs = np.stack([results[c]["out"] for c in range(NCORES)])  # [8, NP, L, D]
    out = outs.reshape(B, H, L, D).transpose(0, 2, 1, 3)
    return np.ascontiguousarray(out)


def run_sharded(queries, keys, values, **kw):
    """Run on the 8 neuron cores; returns (full_output, BassKernelResults)."""
    nc = _build()
    in_maps = _prep_inputs(queries, keys, values)
    res = run_bass_kernel_spmd(nc, in_maps, list(range(NCORES)), **kw)
    return _gather(res.results), res


def kernel(queries, keys, values):
    out, _ = run_sharded(queries, keys, values)
    return out


# revision 15
# speedup vs baseline: 1.1000x; 1.1000x over previous
"""Trainium2 Bass kernel for DynamicSparseAttention.

Reference computation (per batch b, head h):
    scores  = Q @ K^T                      [L, S]
    dense   = softmax(scores, axis=-1)
    routing = dense ** 5
    combined = (routing + dense) * 0.5
    sparse  = combined / sum(combined, -1, keepdims=True)
    out     = sparse @ V                   [L, D]

Math: with p = exp(s - m), Z = sum_s p (any per-row shift m),
    out = (P5 @ V + Z^4 * (P @ V)) / (W + Z^5),  W = sum_s p^5.
A ones-column appended to V makes the matmul accumulators carry the row sums:
    B = P @ [V|1]  -> B[:, D] = Z;   A = P5 @ [V|1] -> A[:, D] = W.

Numerical strategy (two rounds, unconditionally stable):
  Round A uses a loose per-row shift m1 = sqrt(2 ln S)|q_l| + 25 (only needs
  to be within ~+-80 of the true row max).  Its ones-column yields Z_A, i.e.
  the true per-row logsumexp m2 = m1 + ln Z_A.  Round B recomputes scores
  shifted by m2 (so sum exp(s - m2) == 1) and accumulates A = P5 @ [V|1]
  with p5 = exp(5(s - m2)) in [e^-38, 1] — no overflow/underflow possible.
  With Z_2 == 1 the result collapses to
    out = (A + B/Z_A)[:, :D] / (A + B/Z_A)[:, D].

Both shifts are folded into the QK^T matmul as a 65th contraction channel:
  Q_aug = [q | -m],  K_aug = [k | 1]
Round A runs with m = m1; between rounds the shift row is updated in place
to -m2 = -m1 - ln Z_A (ACT Ln on the PSUM accumulator row, then a DVE
subtract, all on partition 64 — the next l-half's columns are untouched),
and round B reruns the same matmul shape with the corrected shift.

Layout: scores are computed transposed, [s partitions, l free], so the
exp() outputs feed the P@V' matmuls directly (contraction over s on
partitions, V' stationary).  Matmuls run as float32r (full-rate fp32).

Sharding: B*H = 32 (b,h) pairs, 4 per core across 8 cores, no cross-core
communication.  kernel() takes full inputs and returns the full output.
"""

import os
import sys
import numpy as np

for _p in ("/opt/trn_rl_repo",):
    if os.path.isdir(_p) and _p not in sys.path:
        sys.path.insert(0, _p)

from contextlib import ExitStack

import json as _json

import concourse.bass as bass
import concourse.mybir as mybir
import concourse.tile as tile
import concourse.bass2jax as _bass2jax
import concourse.bass_utils as _bass_utils
from concourse.bass_utils import run_bass_kernel_spmd
from concourse.masks import make_identity

# ---------------------------------------------------------------------------
# Workaround: this container's walrus build rejects instructions carrying
# more than one sync wait ("Too many sync wait commands", CoreV3GenImpl
# setupSyncWait<...>).  Tile's scheduler freely attaches 2-3 waits per
# instruction (and ~27 on the tail drain).  Rewrite the BIR JSON before
# compilation: excess waits are hoisted onto freshly inserted same-engine
# NoOp instructions placed immediately before the instruction, one wait
# each.  Semantics are unchanged (waits are conjunctive >= conditions and
# engine program order is preserved).
# ---------------------------------------------------------------------------

_MAX_WAITS = 1


def _split_waits_in_bir(bir_json: bytes) -> bytes:
    bir = _json.loads(bir_json)
    n_new = [0]

    def fix_block(bb):
        out = []
        for inst in bb["instructions"]:
            si = inst.get("sync_info") or {}
            waits = si.get("on_wait") or []
            if len(waits) > _MAX_WAITS:
                excess, keep = waits[:-_MAX_WAITS], waits[-_MAX_WAITS:]
                for w in excess:
                    n_new[0] += 1
                    out.append({
                        "debug": inst.get("debug", 0),
                        "engine": inst["engine"],
                        "ins": [],
                        "name": "I-wsplit-%d" % n_new[0],
                        "opcode": "NoOp",
                        "outs": [],
                        "sync_info": {"on_update": [], "on_wait": [w]},
                    })
                si["on_wait"] = keep
            out.append(inst)
        bb["instructions"] = out

    for fn in bir["functions"]:
        for bb in fn["blocks"]:
            fix_block(bb)
    return _json.dumps(bir).encode()


_orig_compile_bir_kernel = _bass_utils.compile_bir_kernel


def _patched_compile_bir_kernel(bir_json, tmpdir, neff_name="file.neff"):
    return _orig_compile_bir_kernel(
        _split_waits_in_bir(bir_json), tmpdir, neff_name=neff_name
    )


_bass_utils.compile_bir_kernel = _patched_compile_bir_kernel
_bass2jax.compile_bir_kernel = _patched_compile_bir_kernel

B, L, S, H, E, D = 2, 2048, 2048, 16, 64, 64
NCORES = 8
NP = (B * H) // NCORES  # pairs per core = 4
EA = E + 1  # q/k channels: 64 data + shift row
DA = D + 1  # v columns: 64 data + ones column
LT = L // 128
ST = S // 128
LHALF = 1024  # l columns per accumulation pass (PSUM capacity)
NCH = 2  # 512-wide matmul chunks per l-half
FACTOR = 5.0

F32 = mybir.dt.float32
F32R = mybir.dt.float32r
EXP = mybir.ActivationFunctionType.Exp
LN = mybir.ActivationFunctionType.Ln

M_COEF = float(np.sqrt(2.0 * np.log(S)))
M_MARGIN = 25.0


def _r(ap):
    return ap.bitcast(F32R)


def _emit(ctx: ExitStack, tc: tile.TileContext, qa, ka, va, outp):
    nc = tc.nc

    const = ctx.enter_context(tc.tile_pool(name="const", bufs=1))
    nat = ctx.enter_context(tc.tile_pool(name="nat", bufs=4))
    big = ctx.enter_context(tc.tile_pool(name="big", bufs=2))
    vpool = ctx.enter_context(tc.tile_pool(name="vp", bufs=2))
    ppool = ctx.enter_context(tc.tile_pool(name="pp", bufs=4))
    eppool = ctx.enter_context(tc.tile_pool(name="ep", bufs=2))
    opool = ctx.enter_context(tc.tile_pool(name="op", bufs=4))
    zpool = ctx.enter_context(tc.tile_pool(name="zp", bufs=2))

    ps_sc = ctx.enter_context(tc.tile_pool(name="ps_sc", bufs=2, space="PSUM"))
    ps_acc = ctx.enter_context(tc.tile_pool(name="ps_acc", bufs=1, space="PSUM"))

    ident = const.tile([128, 128], F32)
    make_identity(nc, ident)
    ident65 = const.tile([DA, DA], F32)
    make_identity(nc, ident65)

    for bh in range(NP):
        # ---- setup: load Q/K tiles, transpose to [EA, L] via PE ----
        qt = big.tile([EA, L], F32R, tag="qt")
        kt = big.tile([EA, S], F32R, tag="kt")
        for t in range(LT):
            qn = nat.tile([128, EA], F32, tag="nat")
            nc.sync.dma_start(out=qn, in_=qa[bh, t * 128:(t + 1) * 128, :])
            qt_ps = ps_sc.tile([EA, 128], F32, tag="sc", name="qt_ps")
            nc.tensor.transpose(qt_ps, qn, ident)
            nc.vector.tensor_copy(qt[:, t * 128:(t + 1) * 128], qt_ps)
        for t in range(ST):
            kn = nat.tile([128, EA], F32, tag="nat")
            nc.sync.dma_start(out=kn, in_=ka[bh, t * 128:(t + 1) * 128, :])
            kt_ps = ps_sc.tile([EA, 128], F32, tag="sc", name="kt_ps")
            nc.tensor.transpose(kt_ps, kn, ident)
            nc.vector.tensor_copy(kt[:, t * 128:(t + 1) * 128], kt_ps)

        vts = []
        for t in range(ST):
            vt = vpool.tile([128, DA], F32R, tag=f"v{t}", name=f"vt{t}")
            nc.sync.dma_start(out=vt, in_=va[bh, t * 128:(t + 1) * 128, :])
            vts.append(vt)

        # ---- main: round A for both l-halves, then round B, then epilogue.
        # Ordering A(lh0), A(lh1), B(lh0), B(lh1) hides the mid-phase
        # (Ln/shift-update) latency of each l-half behind the other's
        # matmul work, and lets the 4 PSUM accumulator banks be reused
        # between rounds (B accumulators are copied to SBUF right after
        # round A so the slots free up for round B).
        NLH = L // LHALF
        b_sbs, a_sbs = {}, {}
        for lh in range(NLH):
            l0 = lh * LHALF
            accb = [ps_acc.tile([DA, 512], F32, tag="acc%d" % c, name="accb")
                    for c in range(NCH)]
            # round A: p = exp(s - m1); accumulate B = P @ [V|1]
            for st in range(ST):
                sc = ps_sc.tile([128, LHALF], F32, tag="sc", name="scA")
                for c in range(NCH):
                    nc.tensor.matmul(
                        sc[:, c * 512:(c + 1) * 512],
                        lhsT=kt[:, st * 128:(st + 1) * 128],
                        rhs=qt[:, l0 + c * 512: l0 + (c + 1) * 512],
                        start=True, stop=True,
                    )
                p = ppool.tile([128, LHALF], F32R, tag="p", name="p")
                nc.scalar.activation(p, sc, EXP, bias=0.0, scale=1.0)
                for c in range(NCH):
                    nc.tensor.matmul(
                        accb[c], lhsT=vts[st],
                        rhs=p[:, c * 512:(c + 1) * 512],
                        start=(st == 0), stop=(st == ST - 1),
                    )

            # mid: shift row of qt (partition 64, this l-half's columns)
            # updated in place: -m2 = -m1 - ln Z_A; copy B to SBUF to free
            # the accumulator banks for round B.
            zrow = zpool.tile([E + 1, LHALF], F32, tag="zrow", name="zrow")
            b_sb = eppool.tile([DA, LHALF], F32, tag="b_sb%d" % lh, name="b_sb")
            for c in range(NCH):
                cs = slice(c * 512, (c + 1) * 512)
                nc.scalar.activation(zrow[E:E + 1, cs], accb[c][D:DA, :], LN,
                                     bias=0.0, scale=1.0)
                nc.vector.tensor_copy(b_sb[:, cs], accb[c])
            nc.vector.tensor_sub(qt[E:E + 1, l0:l0 + LHALF],
                                 qt[E:E + 1, l0:l0 + LHALF],
                                 zrow[E:E + 1, :])
            b_sbs[lh] = b_sb

        for lh in range(NLH):
            l0 = lh * LHALF
            acca = [ps_acc.tile([DA, 512], F32, tag="acc%d" % c, name="acca")
                    for c in range(NCH)]
            # round B: p5 = exp(5(s - m2)); accumulate A = P5 @ [V|1]
            for st in range(ST):
                sc = ps_sc.tile([128, LHALF], F32, tag="sc", name="scB")
                for c in range(NCH):
                    nc.tensor.matmul(
                        sc[:, c * 512:(c + 1) * 512],
                        lhsT=kt[:, st * 128:(st + 1) * 128],
                        rhs=qt[:, l0 + c * 512: l0 + (c + 1) * 512],
                        start=True, stop=True,
                    )
                p5 = ppool.tile([128, LHALF], F32R, tag="p5", name="p5")
                nc.scalar.activation(p5, sc, EXP, bias=0.0, scale=FACTOR)
                for c in range(NCH):
                    nc.tensor.matmul(
                        acca[c], lhsT=vts[st],
                        rhs=p5[:, c * 512:(c + 1) * 512],
                        start=(st == 0), stop=(st == ST - 1),
                    )
            a_sb = eppool.tile([DA, LHALF], F32, tag="a_sb%d" % lh, name="a_sb")
            for c in range(NCH):
                cs = slice(c * 512, (c + 1) * 512)
                nc.vector.tensor_copy(a_sb[:, cs], acca[c])
            a_sbs[lh] = a_sb

        # ---- epilogue: out = (A + B/Z_A)[:, :D] / (A + B/Z_A)[:, D] ----
        for lh in range(NLH):
            l0 = lh * LHALF
            a_sb, b_sb = a_sbs[lh], b_sbs[lh]
            for ch in range(LHALF // 128):
                at_ps = ps_sc.tile([128, DA], F32, tag="sc", name="at_ps")
                bt_ps = ps_sc.tile([128, DA], F32, tag="sc", name="bt_ps")
                nc.tensor.transpose(at_ps, a_sb[:, ch * 128:(ch + 1) * 128],
                                    ident65)
                nc.tensor.transpose(bt_ps, b_sb[:, ch * 128:(ch + 1) * 128],
                                    ident65)
                z = zpool.tile([128, 4], F32, tag="z", name="z")
                nc.vector.reciprocal(z[:, 0:1], bt_ps[:, D:DA])  # 1/Z_A
                n65 = opool.tile([128, DA], F32, tag="n65", name="n65")
                nc.vector.tensor_scalar_mul(n65, bt_ps, z[:, 0:1])
                nc.vector.tensor_add(n65, n65, at_ps)
                nc.vector.reciprocal(z[:, 1:2], n65[:, D:DA])    # 1/den
                ot = opool.tile([128, D], F32, tag="ot", name="ot")
                nc.vector.tensor_scalar_mul(ot, n65[:, 0:D], z[:, 1:2])
                lrow = l0 + ch * 128
                nc.gpsimd.dma_start(out=outp[bh, lrow:lrow + 128, :], in_=ot)


_CACHE = {}


def _build():
    if "nc" in _CACHE:
        return _CACHE["nc"]
    nc = bass.Bass()
    qa = nc.declare_dram_parameter("qa", [NP, L, EA], F32, isOutput=False)
    ka = nc.declare_dram_parameter("ka", [NP, S, EA], F32, isOutput=False)
    va = nc.declare_dram_parameter("va", [NP, S, DA], F32R, isOutput=False)
    outp = nc.declare_dram_parameter("out", [NP, L, D], F32, isOutput=True)
    with tile.TileContext(nc) as tc:
        with ExitStack() as ctx:
            _emit(ctx, tc, qa[:], ka[:], va[:], outp[:])
    _CACHE["nc"] = nc
    return nc


def _prep_inputs(queries, keys, values):
    q = np.ascontiguousarray(np.asarray(queries, np.float32).transpose(0, 2, 1, 3)
                             ).reshape(B * H, L, E)
    k = np.ascontiguousarray(np.asarray(keys, np.float32).transpose(0, 2, 1, 3)
                             ).reshape(B * H, S, E)
    v = np.ascontiguousarray(np.asarray(values, np.float32).transpose(0, 2, 1, 3)
                             ).reshape(B * H, S, D)
    m1 = (M_COEF * np.sqrt((q.astype(np.float64) ** 2).sum(-1)) + M_MARGIN
          ).astype(np.float32)  # [BH, L]
    one = np.ones((B * H, S, 1), np.float32)
    qa = np.concatenate([q, -m1[..., None]], axis=-1)
    ka = np.concatenate([k, one], axis=-1)
    va = np.concatenate([v, one], axis=-1)
    in_maps = []
    for c in range(NCORES):
        sl = slice(c * NP, (c + 1) * NP)
        in_maps.append({
            "qa": np.ascontiguousarray(qa[sl]),
            "ka": np.ascontiguousarray(ka[sl]),
            "va": np.ascontiguousarray(va[sl]),
        })
    return in_maps


def _gather(results):
    outs = np.stack([results[c]["out"] for c in range(NCORES)])  # [8, NP, L, D]
    out = outs.reshape(B, H, L, D).transpose(0, 2, 1, 3)
    return np.ascontiguousarray(out)


def run_sharded(queries, keys, values, **kw):
    """Run on the 8 neuron cores; returns (full_output, BassKernelResults)."""
    nc = _build()
    in_maps = _prep_inputs(queries, keys, values)
    res = run_bass_kernel_spmd(nc, in_maps, list(range(NCORES)), **kw)
    return _gather(res.results), res


def kernel(queries, keys, values):
    out, _ = run_sharded(queries, keys, values)
    return out


# revision 16
# speedup vs baseline: 1.1200x; 1.0182x over previous
"""Trainium2 Bass kernel for DynamicSparseAttention.

Reference computation (per batch b, head h):
    scores  = Q @ K^T                      [L, S]
    dense   = softmax(scores, axis=-1)
    routing = dense ** 5
    combined = (routing + dense) * 0.5
    sparse  = combined / sum(combined, -1, keepdims=True)
    out     = sparse @ V                   [L, D]

Math: with p = exp(s - m), Z = sum_s p (any per-row shift m),
    out = (P5 @ V + Z^4 * (P @ V)) / (W + Z^5),  W = sum_s p^5.
A ones-column appended to V makes the matmul accumulators carry the row sums:
    B = P @ [V|1]  -> B[:, D] = Z;   A = P5 @ [V|1] -> A[:, D] = W.

Numerical strategy (two rounds, unconditionally stable):
  Round A uses a loose per-row shift m1 = sqrt(2 ln S)|q_l| + 25 (only needs
  to be within ~+-80 of the true row max).  Its ones-column yields Z_A, i.e.
  the true per-row logsumexp m2 = m1 + ln Z_A.  Round B recomputes scores
  shifted by m2 (so sum exp(s - m2) == 1) and accumulates A = P5 @ [V|1]
  with p5 = exp(5(s - m2)) in [e^-38, 1] — no overflow/underflow possible.
  With Z_2 == 1 the result collapses to
    out = (A + B/Z_A)[:, :D] / (A + B/Z_A)[:, D].

Both shifts are folded into the QK^T matmul as a 65th contraction channel:
  Q_aug = [q | -m],  K_aug = [k | 1]
Round A runs with m = m1; between rounds the shift row is updated in place
to -m2 = -m1 - ln Z_A (ACT Ln on the PSUM accumulator row, then a DVE
subtract, all on partition 64 — the next l-half's columns are untouched),
and round B reruns the same matmul shape with the corrected shift.

Layout: scores are computed transposed, [s partitions, l free], so the
exp() outputs feed the P@V' matmuls directly (contraction over s on
partitions, V' stationary).  Matmuls run as float32r (full-rate fp32).

Sharding: B*H = 32 (b,h) pairs, 4 per core across 8 cores, no cross-core
communication.  kernel() takes full inputs and returns the full output.
"""

import os
import sys
import numpy as np

for _p in ("/opt/trn_rl_repo",):
    if os.path.isdir(_p) and _p not in sys.path:
        sys.path.insert(0, _p)

from contextlib import ExitStack

import json as _json

import concourse.bass as bass
import concourse.mybir as mybir
import concourse.tile as tile
import concourse.bass2jax as _bass2jax
import concourse.bass_utils as _bass_utils
from concourse.bass_utils import run_bass_kernel_spmd
from concourse.masks import make_identity

# ---------------------------------------------------------------------------
# Workaround: this container's walrus build rejects instructions carrying
# more than one sync wait ("Too many sync wait commands", CoreV3GenImpl
# setupSyncWait<...>).  Tile's scheduler freely attaches 2-3 waits per
# instruction (and ~27 on the tail drain).  Rewrite the BIR JSON before
# compilation: excess waits are hoisted onto freshly inserted same-engine
# NoOp instructions placed immediately before the instruction, one wait
# each.  Semantics are unchanged (waits are conjunctive >= conditions and
# engine program order is preserved).
# ---------------------------------------------------------------------------

_MAX_WAITS = 1


def _split_waits_in_bir(bir_json: bytes) -> bytes:
    bir = _json.loads(bir_json)
    n_new = [0]

    def fix_block(bb):
        out = []
        for inst in bb["instructions"]:
            si = inst.get("sync_info") or {}
            waits = si.get("on_wait") or []
            if len(waits) > _MAX_WAITS:
                excess, keep = waits[:-_MAX_WAITS], waits[-_MAX_WAITS:]
                for w in excess:
                    n_new[0] += 1
                    out.append({
                        "debug": inst.get("debug", 0),
                        "engine": inst["engine"],
                        "ins": [],
                        "name": "I-wsplit-%d" % n_new[0],
                        "opcode": "NoOp",
                        "outs": [],
                        "sync_info": {"on_update": [], "on_wait": [w]},
                    })
                si["on_wait"] = keep
            out.append(inst)
        bb["instructions"] = out

    for fn in bir["functions"]:
        for bb in fn["blocks"]:
            fix_block(bb)
    return _json.dumps(bir).encode()


_orig_compile_bir_kernel = _bass_utils.compile_bir_kernel


def _patched_compile_bir_kernel(bir_json, tmpdir, neff_name="file.neff"):
    return _orig_compile_bir_kernel(
        _split_waits_in_bir(bir_json), tmpdir, neff_name=neff_name
    )


_bass_utils.compile_bir_kernel = _patched_compile_bir_kernel
_bass2jax.compile_bir_kernel = _patched_compile_bir_kernel

B, L, S, H, E, D = 2, 2048, 2048, 16, 64, 64
NCORES = 8
NP = (B * H) // NCORES  # pairs per core = 4
EA = E + 1  # q/k channels: 64 data + shift row
DA = D + 1  # v columns: 64 data + ones column
LT = L // 128
ST = S // 128
LHALF = 1024  # l columns per accumulation pass (PSUM capacity)
NCH = 2  # 512-wide matmul chunks per l-half
FACTOR = 5.0

F32 = mybir.dt.float32
F32R = mybir.dt.float32r
BF16 = mybir.dt.bfloat16
EXP = mybir.ActivationFunctionType.Exp
LN = mybir.ActivationFunctionType.Ln

M_COEF = float(np.sqrt(2.0 * np.log(S)))
M_MARGIN = 25.0


def _r(ap):
    return ap.bitcast(F32R)


def _emit(ctx: ExitStack, tc: tile.TileContext, qa, ka, va, outp):
    nc = tc.nc

    const = ctx.enter_context(tc.tile_pool(name="const", bufs=1))
    nat = ctx.enter_context(tc.tile_pool(name="nat", bufs=4))
    big = ctx.enter_context(tc.tile_pool(name="big", bufs=2))
    vpool = ctx.enter_context(tc.tile_pool(name="vp", bufs=2))
    ppool = ctx.enter_context(tc.tile_pool(name="pp", bufs=4))
    eppool = ctx.enter_context(tc.tile_pool(name="ep", bufs=2))
    opool = ctx.enter_context(tc.tile_pool(name="op", bufs=4))
    zpool = ctx.enter_context(tc.tile_pool(name="zp", bufs=2))

    ps_sc = ctx.enter_context(tc.tile_pool(name="ps_sc", bufs=2, space="PSUM"))
    ps_acc = ctx.enter_context(tc.tile_pool(name="ps_acc", bufs=1, space="PSUM"))

    ident = const.tile([128, 128], F32)
    make_identity(nc, ident)
    ident65 = const.tile([DA, DA], F32)
    make_identity(nc, ident65)

    for bh in range(NP):
        # ---- setup: load Q/K tiles, transpose to [EA, L] via PE ----
        qt = big.tile([EA, L], F32R, tag="qt")
        kt = big.tile([EA, S], F32R, tag="kt")
        for t in range(LT):
            qn = nat.tile([128, EA], F32, tag="nat")
            nc.sync.dma_start(out=qn, in_=qa[bh, t * 128:(t + 1) * 128, :])
            qt_ps = ps_sc.tile([EA, 128], F32, tag="sc", name="qt_ps")
            nc.tensor.transpose(qt_ps, qn, ident)
            nc.vector.tensor_copy(qt[:, t * 128:(t + 1) * 128], qt_ps)
        for t in range(ST):
            kn = nat.tile([128, EA], F32, tag="nat")
            nc.sync.dma_start(out=kn, in_=ka[bh, t * 128:(t + 1) * 128, :])
            kt_ps = ps_sc.tile([EA, 128], F32, tag="sc", name="kt_ps")
            nc.tensor.transpose(kt_ps, kn, ident)
            nc.vector.tensor_copy(kt[:, t * 128:(t + 1) * 128], kt_ps)

        vts = []
        for t in range(ST):
            vt = vpool.tile([128, DA], BF16, tag=f"v{t}", name=f"vt{t}")
            nc.sync.dma_start(out=vt, in_=va[bh, t * 128:(t + 1) * 128, :])
            vts.append(vt)

        # ---- main: round A for both l-halves, then round B, then epilogue.
        # Ordering A(lh0), A(lh1), B(lh0), B(lh1) hides the mid-phase
        # (Ln/shift-update) latency of each l-half behind the other's
        # matmul work, and lets the 4 PSUM accumulator banks be reused
        # between rounds (B accumulators are copied to SBUF right after
        # round A so the slots free up for round B).
        NLH = L // LHALF
        b_sbs, a_sbs = {}, {}
        for lh in range(NLH):
            l0 = lh * LHALF
            accb = [ps_acc.tile([DA, 512], F32, tag="acc%d" % c, name="accb")
                    for c in range(NCH)]
            # round A: p = exp(s - m1); accumulate B = P @ [V|1]
            for st in range(ST):
                sc = ps_sc.tile([128, LHALF], F32, tag="sc", name="scA")
                for c in range(NCH):
                    nc.tensor.matmul(
                        sc[:, c * 512:(c + 1) * 512],
                        lhsT=kt[:, st * 128:(st + 1) * 128],
                        rhs=qt[:, l0 + c * 512: l0 + (c + 1) * 512],
                        start=True, stop=True,
                    )
                p = ppool.tile([128, LHALF], BF16, tag="p", name="p")
                nc.scalar.activation(p, sc, EXP, bias=0.0, scale=1.0)
                for c in range(NCH):
                    nc.tensor.matmul(
                        accb[c], lhsT=vts[st],
                        rhs=p[:, c * 512:(c + 1) * 512],
                        start=(st == 0), stop=(st == ST - 1),
                    )

            # mid: shift row of qt (partition 64, this l-half's columns)
            # updated in place: -m2 = -m1 - ln Z_A; copy B to SBUF to free
            # the accumulator banks for round B.
            zrow = zpool.tile([E + 1, LHALF], F32, tag="zrow", name="zrow")
            b_sb = eppool.tile([DA, LHALF], F32, tag="b_sb%d" % lh, name="b_sb")
            for c in range(NCH):
                cs = slice(c * 512, (c + 1) * 512)
                nc.scalar.activation(zrow[E:E + 1, cs], accb[c][D:DA, :], LN,
                                     bias=0.0, scale=1.0)
                nc.vector.tensor_copy(b_sb[:, cs], accb[c])
            nc.vector.tensor_sub(qt[E:E + 1, l0:l0 + LHALF],
                                 qt[E:E + 1, l0:l0 + LHALF],
                                 zrow[E:E + 1, :])
            b_sbs[lh] = b_sb

        for lh in range(NLH):
            l0 = lh * LHALF
            acca = [ps_acc.tile([DA, 512], F32, tag="acc%d" % c, name="acca")
                    for c in range(NCH)]
            # round B: p5 = exp(5(s - m2)); accumulate A = P5 @ [V|1]
            for st in range(ST):
                sc = ps_sc.tile([128, LHALF], F32, tag="sc", name="scB")
                for c in range(NCH):
                    nc.tensor.matmul(
                        sc[:, c * 512:(c + 1) * 512],
                        lhsT=kt[:, st * 128:(st + 1) * 128],
                        rhs=qt[:, l0 + c * 512: l0 + (c + 1) * 512],
                        start=True, stop=True,
                    )
                p5 = ppool.tile([128, LHALF], BF16, tag="p5", name="p5")
                nc.scalar.activation(p5, sc, EXP, bias=0.0, scale=FACTOR)
                for c in range(NCH):
                    nc.tensor.matmul(
                        acca[c], lhsT=vts[st],
                        rhs=p5[:, c * 512:(c + 1) * 512],
                        start=(st == 0), stop=(st == ST - 1),
                    )
            a_sb = eppool.tile([DA, LHALF], F32, tag="a_sb%d" % lh, name="a_sb")
            for c in range(NCH):
                cs = slice(c * 512, (c + 1) * 512)
                nc.vector.tensor_copy(a_sb[:, cs], acca[c])
            a_sbs[lh] = a_sb

        # ---- epilogue: out = (A + B/Z_A)[:, :D] / (A + B/Z_A)[:, D] ----
        for lh in range(NLH):
            l0 = lh * LHALF
            a_sb, b_sb = a_sbs[lh], b_sbs[lh]
            for ch in range(LHALF // 128):
                at_ps = ps_sc.tile([128, DA], F32, tag="sc", name="at_ps")
                bt_ps = ps_sc.tile([128, DA], F32, tag="sc", name="bt_ps")
                nc.tensor.transpose(at_ps, a_sb[:, ch * 128:(ch + 1) * 128],
                                    ident65)
                nc.tensor.transpose(bt_ps, b_sb[:, ch * 128:(ch + 1) * 128],
                                    ident65)
                z = zpool.tile([128, 4], F32, tag="z", name="z")
                nc.vector.reciprocal(z[:, 0:1], bt_ps[:, D:DA])  # 1/Z_A
                n65 = opool.tile([128, DA], F32, tag="n65", name="n65")
                nc.vector.tensor_scalar_mul(n65, bt_ps, z[:, 0:1])
                nc.vector.tensor_add(n65, n65, at_ps)
                nc.vector.reciprocal(z[:, 1:2], n65[:, D:DA])    # 1/den
                ot = opool.tile([128, D], F32, tag="ot", name="ot")
                nc.vector.tensor_scalar_mul(ot, n65[:, 0:D], z[:, 1:2])
                lrow = l0 + ch * 128
                nc.gpsimd.dma_start(out=outp[bh, lrow:lrow + 128, :], in_=ot)


_CACHE = {}


def _build():
    if "nc" in _CACHE:
        return _CACHE["nc"]
    nc = bass.Bass()
    qa = nc.declare_dram_parameter("qa", [NP, L, EA], F32, isOutput=False)
    ka = nc.declare_dram_parameter("ka", [NP, S, EA], F32, isOutput=False)
    va = nc.declare_dram_parameter("va", [NP, S, DA], BF16, isOutput=False)
    outp = nc.declare_dram_parameter("out", [NP, L, D], F32, isOutput=True)
    with tile.TileContext(nc) as tc:
        with ExitStack() as ctx:
            _emit(ctx, tc, qa[:], ka[:], va[:], outp[:])
    _CACHE["nc"] = nc
    return nc


def _prep_inputs(queries, keys, values):
    q = np.ascontiguousarray(np.asarray(queries, np.float32).transpose(0, 2, 1, 3)
                             ).reshape(B * H, L, E)
    k = np.ascontiguousarray(np.asarray(keys, np.float32).transpose(0, 2, 1, 3)
                             ).reshape(B * H, S, E)
    v = np.ascontiguousarray(np.asarray(values, np.float32).transpose(0, 2, 1, 3)
                             ).reshape(B * H, S, D)
    m1 = (M_COEF * np.sqrt((q.astype(np.float64) ** 2).sum(-1)) + M_MARGIN
          ).astype(np.float32)  # [BH, L]
    one = np.ones((B * H, S, 1), np.float32)
    qa = np.concatenate([q, -m1[..., None]], axis=-1)
    ka = np.concatenate([k, one], axis=-1)
    va = np.concatenate([v, one], axis=-1)
    import ml_dtypes
    va = va.astype(ml_dtypes.bfloat16)
    in_maps = []
    for c in range(NCORES):
        sl = slice(c * NP, (c + 1) * NP)
        in_maps.append({
            "qa": np.ascontiguousarray(qa[sl]),
            "ka": np.ascontiguousarray(ka[sl]),
            "va": np.ascontiguousarray(va[sl]),
        })
    return in_maps


def _gather(results):
    outs = np.stack([results[c]["out"] for c in range(NCORES)])  # [8, NP, L, D]
    out = outs.reshape(B, H, L, D).transpose(0, 2, 1, 3)
    return np.ascontiguousarray(out)


def run_sharded(queries, keys, values, **kw):
    """Run on the 8 neuron cores; returns (full_output, BassKernelResults)."""
    nc = _build()
    in_maps = _prep_inputs(queries, keys, values)
    res = run_bass_kernel_spmd(nc, in_maps, list(range(NCORES)), **kw)
    return _gather(res.results), res


def kernel(queries, keys, values):
    out, _ = run_sharded(queries, keys, values)
    return out
